# revision 5
# baseline (speedup 1.0000x reference)
"""Causal self-attention (B=8, T=1500, C=256, H=8, D=32) on 8 trn2 NeuronCores.

Sharding: data-parallel over batch B — core b computes batch element b
end-to-end (no collectives). The host only re-lays-out inputs (transposes /
replication); every FLOP of the module runs on device.

Per-core device algorithm:
  - q^T, k^T computed in transposed orientation [C, T] so each head h lives
    on partitions 32h..32h+31 (exactly what the score matmuls need as K=32
    contractions); v computed in natural orientation [T, C] (it is the PV
    lhsT directly).
  - Scores are computed TRANSPOSED per head: S^T[k, q] with k on partitions,
    4 heads packed into one PE pass via tile_position row-tiling (K=32 each)
    into 4 adjacent PSUM banks.
  - exp runs on the scalar engine straight PSUM->SBUF with the 1/sqrt(D)
    scale folded in; no max-subtraction (scores are O(1) by construction so
    exp cannot overflow in fp32).
  - Causal masking multiplies only the single diagonal-crossing 128-column
    strip by a 0/1 lower-triangle mask; fully-masked columns are simply
    never computed (matmul N-ranges start at the diagonal).
  - P@V col-tiles 4 heads (M=32 each) accumulating into one PSUM bank that
    lands heads at partitions 32h — the y^T slab layout. Softmax
    denominators use an all-ones [128, 32] lhsT so each head's denominator
    is replicated across its 32 partitions for free.
  - Normalization: reciprocal_approx_fast + one tensor_tensor multiply
    (PSUM -> y^T SBUF slab). Output projection uses y^T as lhsT with Wp^T
    as rhs -> natural [T, C] tiles -> bias add -> DMA out.
"""

import numpy as np

B, T, C = 8, 1500, 256
H, D = 8, 32
GH = 4  # heads per group (two groups of 4)
SCALE = 1.0 / float(np.sqrt(D))
N_CORES = 8

Q_TILES = [(0, 512), (512, 512), (1024, 476)]
K_TILES = [(j * 128, min(128, T - j * 128)) for j in range(12)]
T_TILES = K_TILES  # same tiling for t-dim work (v-proj / out-proj)

_CACHE = {}


def _build():
    import concourse.bass as bass
    import concourse.mybir as mybir
    import concourse.tile as tile
    from concourse import bacc

    f32 = mybir.dt.float32
    AF = mybir.ActivationFunctionType
    ALU = mybir.AluOpType

    nc = bacc.Bacc()

    xt_d = nc.dram_tensor("xt", [C, T], f32, kind="ExternalInput")
    wqt_d = nc.dram_tensor("wqt", [C, C], f32, kind="ExternalInput")
    wkt_d = nc.dram_tensor("wkt", [C, C], f32, kind="ExternalInput")
    wvt_d = nc.dram_tensor("wvt", [C, C], f32, kind="ExternalInput")
    wpt_d = nc.dram_tensor("wpt", [C, C], f32, kind="ExternalInput")
    bq_d = nc.dram_tensor("bq2", [128, 2], f32, kind="ExternalInput")
    bk_d = nc.dram_tensor("bk2", [128, 2], f32, kind="ExternalInput")
    bv_d = nc.dram_tensor("bvrep", [128, C], f32, kind="ExternalInput")
    bp_d = nc.dram_tensor("bprep", [128, C], f32, kind="ExternalInput")
    msk_d = nc.dram_tensor("bmask4", [128, 4, 128], f32, kind="ExternalInput")
    ones_d = nc.dram_tensor("ones32", [128, 32], f32, kind="ExternalInput")
    out_d = nc.dram_tensor("out", [T, C], f32, kind="ExternalOutput")

    from contextlib import ExitStack

    with tile.TileContext(nc) as tc, ExitStack() as stack:
        # ---------------- persistent SBUF tiles ----------------
        pp = stack.enter_context(tc.tile_pool(name="persist", bufs=1))
        xt0 = pp.tile([128, T], f32, name="xt0")
        xt1 = pp.tile([128, T], f32, name="xt1")
        xt = [xt0, xt1]
        wqt0 = pp.tile([128, C], f32, name="wqt0")
        wqt1 = pp.tile([128, C], f32, name="wqt1")
        wkt0 = pp.tile([128, C], f32, name="wkt0")
        wkt1 = pp.tile([128, C], f32, name="wkt1")
        wvt0 = pp.tile([128, C], f32, name="wvt0")
        wvt1 = pp.tile([128, C], f32, name="wvt1")
        wpt0 = pp.tile([128, C], f32, name="wpt0")
        wpt1 = pp.tile([128, C], f32, name="wpt1")
        wqt, wkt, wvt, wpt = [wqt0, wqt1], [wkt0, wkt1], [wvt0, wvt1], [wpt0, wpt1]
        bq_s = pp.tile([128, 2], f32, name="bq_s")
        bk_s = pp.tile([128, 2], f32, name="bk_s")
        bv_s = pp.tile([128, C], f32, name="bv_s")
        bp_s = pp.tile([128, C], f32, name="bp_s")
        msk_s = pp.tile([128, 4, 128], f32, name="msk_s")
        ones_s = pp.tile([128, 32], f32, name="ones_s")
        qt0 = pp.tile([128, T], f32, name="qt0")
        qt1 = pp.tile([128, T], f32, name="qt1")
        kt0 = pp.tile([128, T], f32, name="kt0")
        kt1 = pp.tile([128, T], f32, name="kt1")
        qt, kt = [qt0, qt1], [kt0, kt1]
        vnat = pp.tile([128, 12 * C], f32, name="vnat")
        yt0 = pp.tile([128, T], f32, name="yt0")
        yt1 = pp.tile([128, T], f32, name="yt1")
        yt = [yt0, yt1]
        warm = pp.tile([128, 8], f32, name="warm")

        # ---------------- input DMAs ----------------
        for i in range(2):
            nc.gpsimd.dma_start(out=xt[i], in_=xt_d[i * 128 : (i + 1) * 128, :])
            nc.gpsimd.dma_start(out=wqt[i], in_=wqt_d[i * 128 : (i + 1) * 128, :])
            nc.gpsimd.dma_start(out=wkt[i], in_=wkt_d[i * 128 : (i + 1) * 128, :])
            nc.gpsimd.dma_start(out=wvt[i], in_=wvt_d[i * 128 : (i + 1) * 128, :])
            nc.gpsimd.dma_start(out=wpt[i], in_=wpt_d[i * 128 : (i + 1) * 128, :])
        nc.gpsimd.dma_start(out=bq_s, in_=bq_d[:, :])
        nc.gpsimd.dma_start(out=bk_s, in_=bk_d[:, :])
        nc.gpsimd.dma_start(out=bv_s, in_=bv_d[:, :])
        nc.gpsimd.dma_start(out=bp_s, in_=bp_d[:, :])
        nc.gpsimd.dma_start(out=msk_s, in_=msk_d[:, :, :])
        nc.gpsimd.dma_start(out=ones_s, in_=ones_d[:, :])

        # warm up the ACT exp table set before the real exps need it
        nc.vector.memset(warm[:, 0:4], 0.0)
        nc.scalar.activation(warm[:, 4:8], warm[:, 0:4], AF.Exp)

        # ---------------- phase 1: projections ----------------
        with tc.tile_pool(name="proj_psum", bufs=2, space="PSUM") as psA:
            # q^T and k^T in transposed orientation: slab m holds heads
            # 4m..4m+3 (c_out rows m*128..m*128+127) over all T columns.
            for n, (n0, nn) in enumerate(Q_TILES):
                for m in range(2):
                    qp = psA.tile([128, 512], f32, name="qp", tag="qp")
                    for kk in range(2):
                        nc.tensor.matmul(
                            out=qp[:, 0:nn],
                            lhsT=wqt[kk][:, m * 128 : (m + 1) * 128],
                            rhs=xt[kk][:, n0 : n0 + nn],
                            start=(kk == 0),
                            stop=(kk == 1),
                        )
                    nc.vector.tensor_scalar_add(
                        out=qt[m][:, n0 : n0 + nn],
                        in0=qp[:, 0:nn],
                        scalar1=bq_s[:, m : m + 1],
                    )
                    kp = psA.tile([128, 512], f32, name="kp", tag="kp")
                    for kk in range(2):
                        nc.tensor.matmul(
                            out=kp[:, 0:nn],
                            lhsT=wkt[kk][:, m * 128 : (m + 1) * 128],
                            rhs=xt[kk][:, n0 : n0 + nn],
                            start=(kk == 0),
                            stop=(kk == 1),
                        )
                    nc.vector.tensor_scalar_add(
                        out=kt[m][:, n0 : n0 + nn],
                        in0=kp[:, 0:nn],
                        scalar1=bk_s[:, m : m + 1],
                    )
            # v in natural orientation [T, C]: block tt covers t rows
            # tt*128..tt*128+127 at vnat columns tt*256..tt*256+255.
            for tt, (t0, tl) in enumerate(T_TILES):
                vp = psA.tile([128, C], f32, name="vp", tag="vp")
                for kk in range(2):
                    nc.tensor.matmul(
                        out=vp[0:tl, :],
                        lhsT=xt[kk][:, t0 : t0 + tl],
                        rhs=wvt[kk][:, :],
                        start=(kk == 0),
                        stop=(kk == 1),
                    )
                nc.vector.tensor_tensor(
                    out=vnat[0:tl, tt * C : (tt + 1) * C],
                    in0=vp[0:tl, :],
                    in1=bv_s[0:tl, :],
                    op=ALU.add,
                )

        # ---------------- phase 2: attention + output projection ----------------
        with (
            tc.tile_pool(name="s_psum", bufs=1, space="PSUM") as psS,
            tc.tile_pool(name="y_psum", bufs=1, space="PSUM") as psY,
            tc.tile_pool(name="d_psum", bufs=1, space="PSUM") as psD,
            tc.tile_pool(name="o_psum", bufs=2, space="PSUM") as psO,
            tc.tile_pool(name="eslab", bufs=3) as psE,
            tc.tile_pool(name="rrec", bufs=2) as psR,
            tc.tile_pool(name="ostage", bufs=3) as psT,
        ):
            for qi, (q0, qn) in enumerate(Q_TILES):
                for g in range(2):
                    y_ps = psY.tile([128, 512], f32, name="y_ps", tag="y")
                    d_ps = psD.tile([128, 512], f32, name="d_ps", tag="d")
                    # zero data; all accumulating matmuls below use
                    # start=False so stale has_written bits can't drop
                    # contributions regardless of clear semantics.
                    nc.vector.memset(y_ps[:, 0:qn], 0.0)
                    nc.vector.memset(d_ps[:, 0:qn], 0.0)

                    js = [j for j, (k0, kn) in enumerate(K_TILES) if k0 <= q0 + qn - 1]
                    jlast = js[-1]
                    for j in js:
                        k0, kn = K_TILES[j]
                        r = max(0, k0 - q0)
                        s4 = psS.tile([128, 4, 512], f32, name="s4", tag="s4")
                        for hh in range(GH):
                            nc.tensor.matmul(
                                out=s4[0:kn, hh, r:qn],
                                lhsT=kt[g][32 * hh : 32 * (hh + 1), k0 : k0 + kn],
                                rhs=qt[g][32 * hh : 32 * (hh + 1), q0 + r : q0 + qn],
                                start=True,
                                stop=True,
                                tile_position=(32 * hh, 0),
                            )
                        esl = psE.tile([128, 4, 512], f32, name="esl", tag="esl")
                        nc.scalar.activation(
                            out=esl[0:kn, :, r:qn],
                            in_=s4[0:kn, :, r:qn],
                            func=AF.Exp,
                            scale=SCALE,
                        )
                        if k0 >= q0:  # diagonal-crossing block: 0/1 mask strip
                            w = min(kn, qn - r)
                            nc.vector.tensor_tensor(
                                out=esl[0:kn, :, r : r + w],
                                in0=esl[0:kn, :, r : r + w],
                                in1=msk_s[0:kn, :, 0:w],
                                op=ALU.mult,
                            )
                        for hh in range(GH):
                            nc.tensor.matmul(
                                out=y_ps[32 * hh : 32 * (hh + 1), r:qn],
                                lhsT=vnat[0:kn, j * C + g * 128 + 32 * hh : j * C + g * 128 + 32 * (hh + 1)],
                                rhs=esl[0:kn, hh, r:qn],
                                start=False,
                                stop=(j == jlast),
                                tile_position=(0, 32 * hh),
                                skip_group_check=True,
                            )
                            nc.tensor.matmul(
                                out=d_ps[32 * hh : 32 * (hh + 1), r:qn],
                                lhsT=ones_s[0:kn, :],
                                rhs=esl[0:kn, hh, r:qn],
                                start=False,
                                stop=(j == jlast),
                                tile_position=(0, 32 * hh),
                                skip_group_check=True,
                            )
                    rt = psR.tile([128, 512], f32, name="rt", tag="rt")
                    nc.vector.reciprocal_approx_fast(out=rt[:, 0:qn], in_=d_ps[:, 0:qn])
                    nc.vector.tensor_tensor(
                        out=yt[g][:, q0 : q0 + qn],
                        in0=y_ps[:, 0:qn],
                        in1=rt[:, 0:qn],
                        op=ALU.mult,
                    )
                # output projection for this q-tile's t-range
                for t0 in range(q0, q0 + qn, 128):
                    tl = min(128, q0 + qn - t0)
                    ops = psO.tile([128, C], f32, name="ops", tag="ops")
                    nc.tensor.matmul(
                        out=ops[0:tl, :],
                        lhsT=yt[0][:, t0 : t0 + tl],
                        rhs=wpt[0][:, :],
                        start=True,
                        stop=False,
                    )
                    nc.tensor.matmul(
                        out=ops[0:tl, :],
                        lhsT=yt[1][:, t0 : t0 + tl],
                        rhs=wpt[1][:, :],
                        start=False,
                        stop=True,
                    )
                    ost = psT.tile([128, C], f32, name="ost", tag="ost")
                    nc.vector.tensor_tensor(
                        out=ost[0:tl, :],
                        in0=ops[0:tl, :],
                        in1=bp_s[0:tl, :],
                        op=ALU.add,
                    )
                    nc.sync.dma_start(out=out_d[t0 : t0 + tl, :], in_=ost[0:tl, :])

    nc.compile()
    return nc


def _get_nc():
    if "nc" not in _CACHE:
        _CACHE["nc"] = _build()
    return _CACHE["nc"]


def _make_in_maps(inputs):
    f = np.float32
    x = np.asarray(inputs["x"], f)
    Wq = np.asarray(inputs["Wq"], f)
    Wk = np.asarray(inputs["Wk"], f)
    Wv = np.asarray(inputs["Wv"], f)
    Wp = np.asarray(inputs["Wp"], f)
    bq = np.asarray(inputs["bq"], f)
    bk = np.asarray(inputs["bk"], f)
    bv = np.asarray(inputs["bv"], f)
    bp = np.asarray(inputs["bp"], f)

    tri = np.triu(np.ones((128, 128), f))  # keep where k-row <= q-col
    common = {
        "wqt": np.ascontiguousarray(Wq.T),
        "wkt": np.ascontiguousarray(Wk.T),
        "wvt": np.ascontiguousarray(Wv.T),
        "wpt": np.ascontiguousarray(Wp.T),
        "bq2": np.ascontiguousarray(bq.reshape(2, 128).T),
        "bk2": np.ascontiguousarray(bk.reshape(2, 128).T),
        "bvrep": np.ascontiguousarray(np.tile(bv, (128, 1))),
        "bprep": np.ascontiguousarray(np.tile(bp, (128, 1))),
        "bmask4": np.ascontiguousarray(np.tile(tri[:, None, :], (1, 4, 1))),
        "ones32": np.ones((128, 32), f),
    }
    return [
        {**common, "xt": np.ascontiguousarray(x[b].T)} for b in range(N_CORES)
    ]


def run(inputs, trace=False):
    from concourse.bass_utils import run_bass_kernel_spmd

    nc = _get_nc()
    in_maps = _make_in_maps(inputs)
    res = run_bass_kernel_spmd(nc, in_maps, list(range(N_CORES)), trace=trace)
    out = np.stack([res.results[i]["out"] for i in range(N_CORES)], axis=0)
    return out.astype(np.float32), res


def kernel(**inputs) -> np.ndarray:
    out, _ = run(inputs, trace=False)
    return out


# revision 6
# speedup vs baseline: 1.6067x; 1.6067x over previous
"""Causal self-attention (B=8, T=1500, C=256, H=8, D=32) on 8 trn2 NeuronCores.

Sharding: data-parallel over batch B — core b computes batch element b
end-to-end (no collectives). The host only re-lays-out inputs (transposes /
replication); every FLOP of the module runs on device.

Per-core device algorithm:
  - q^T, k^T computed in transposed orientation [C, T] so each head h lives
    on partitions 32h..32h+31 (what the score matmuls need as K=32
    contractions); v computed in natural orientation [T, C] (it is the PV
    lhsT directly).
  - Scores computed TRANSPOSED per head: S^T[k, q], k on partitions, 4 heads
    per PE pass via tile_position row-tiling into two 2-bank PSUM tiles
    (s4a double-buffered, s4b single) so next k-tile's scores overlap the
    current exp.
  - exp on ScalarE straight PSUM->SBUF with 1/sqrt(D) folded into the
    activation scale, split in two instructions (heads 0-1 / 2-3) so the
    pipeline advances at half-iteration granularity. No max-subtraction:
    scores are O(1) by construction, fp32 exp cannot overflow.
  - Causal masking: multiply only the diagonal-crossing 128-col strip by a
    0/1 lower-triangle mask; fully-masked columns are never computed.
  - P@V col-tiles 4 heads (M=32) into one PSUM bank -> heads land at
    partitions 32h (the y^T slab layout). Denominators via an all-ones
    [128, 32] lhsT: each head's denominator replicated over its 32
    partitions for free.
  - Normalization: reciprocal_approx_fast + one tensor_tensor multiply per
    (group, q-tile). Output projection (deferred tail): y^T as lhsT with
    Wp^T as rhs -> natural [T, C] tiles -> bias add -> DMA out.
"""

import numpy as np

B, T, C = 8, 1500, 256
H, D = 8, 32
GH = 4  # heads per group (two groups of 4)
SCALE = 1.0 / float(np.sqrt(D))
N_CORES = 8

Q_TILES = [(0, 512), (512, 512), (1024, 476)]
K_TILES = [(j * 128, min(128, T - j * 128)) for j in range(12)]
T_TILES = K_TILES  # same tiling for t-dim work (v-proj / out-proj)

_CACHE = {}


def _build():
    import concourse.bass as bass
    import concourse.mybir as mybir
    import concourse.tile as tile
    from concourse import bacc

    f32 = mybir.dt.float32
    AF = mybir.ActivationFunctionType
    ALU = mybir.AluOpType

    nc = bacc.Bacc()

    xt_d = nc.dram_tensor("xt", [C, T], f32, kind="ExternalInput")
    wqt_d = nc.dram_tensor("wqt", [C, C], f32, kind="ExternalInput")
    wkt_d = nc.dram_tensor("wkt", [C, C], f32, kind="ExternalInput")
    wvt_d = nc.dram_tensor("wvt", [C, C], f32, kind="ExternalInput")
    wpt_d = nc.dram_tensor("wpt", [C, C], f32, kind="ExternalInput")
    bq_d = nc.dram_tensor("bq2", [128, 2], f32, kind="ExternalInput")
    bk_d = nc.dram_tensor("bk2", [128, 2], f32, kind="ExternalInput")
    bv_d = nc.dram_tensor("bvrep", [128, C], f32, kind="ExternalInput")
    bp_d = nc.dram_tensor("bprep", [128, C], f32, kind="ExternalInput")
    msk_d = nc.dram_tensor("bmask2", [128, 2, 128], f32, kind="ExternalInput")
    ones_d = nc.dram_tensor("ones32", [128, 32], f32, kind="ExternalInput")
    out_d = nc.dram_tensor("out", [T, C], f32, kind="ExternalOutput")

    from contextlib import ExitStack

    with tile.TileContext(nc) as tc, ExitStack() as stack:
        # ---------------- persistent SBUF tiles ----------------
        pp = stack.enter_context(tc.tile_pool(name="persist", bufs=1))
        xt0 = pp.tile([128, T], f32, name="xt0")
        xt1 = pp.tile([128, T], f32, name="xt1")
        xt = [xt0, xt1]
        wqt0 = pp.tile([128, C], f32, name="wqt0")
        wqt1 = pp.tile([128, C], f32, name="wqt1")
        wkt0 = pp.tile([128, C], f32, name="wkt0")
        wkt1 = pp.tile([128, C], f32, name="wkt1")
        wvt0 = pp.tile([128, C], f32, name="wvt0")
        wvt1 = pp.tile([128, C], f32, name="wvt1")
        wpt0 = pp.tile([128, C], f32, name="wpt0")
        wpt1 = pp.tile([128, C], f32, name="wpt1")
        wqt, wkt, wvt, wpt = [wqt0, wqt1], [wkt0, wkt1], [wvt0, wvt1], [wpt0, wpt1]
        bq_s = pp.tile([128, 2], f32, name="bq_s")
        bk_s = pp.tile([128, 2], f32, name="bk_s")
        bv_s = pp.tile([128, C], f32, name="bv_s")
        bp_s = pp.tile([128, C], f32, name="bp_s")
        msk_s = pp.tile([128, 2, 128], f32, name="msk_s")
        ones_s = pp.tile([128, 32], f32, name="ones_s")
        qt0 = pp.tile([128, T], f32, name="qt0")
        qt1 = pp.tile([128, T], f32, name="qt1")
        kt0 = pp.tile([128, T], f32, name="kt0")
        kt1 = pp.tile([128, T], f32, name="kt1")
        qt, kt = [qt0, qt1], [kt0, kt1]
        vnat = pp.tile([128, 12 * C], f32, name="vnat")
        yt0 = pp.tile([128, T], f32, name="yt0")
        yt1 = pp.tile([128, T], f32, name="yt1")
        yt = [yt0, yt1]
        warm = pp.tile([128, 8], f32, name="warm")

        # ---------------- input DMAs ----------------
        for i in range(2):
            nc.gpsimd.dma_start(out=xt[i], in_=xt_d[i * 128 : (i + 1) * 128, :])
            nc.gpsimd.dma_start(out=wqt[i], in_=wqt_d[i * 128 : (i + 1) * 128, :])
            nc.gpsimd.dma_start(out=wkt[i], in_=wkt_d[i * 128 : (i + 1) * 128, :])
            nc.gpsimd.dma_start(out=wvt[i], in_=wvt_d[i * 128 : (i + 1) * 128, :])
            nc.gpsimd.dma_start(out=wpt[i], in_=wpt_d[i * 128 : (i + 1) * 128, :])
        nc.gpsimd.dma_start(out=bq_s, in_=bq_d[:, :])
        nc.gpsimd.dma_start(out=bk_s, in_=bk_d[:, :])
        nc.gpsimd.dma_start(out=bv_s, in_=bv_d[:, :])
        nc.gpsimd.dma_start(out=bp_s, in_=bp_d[:, :])
        nc.gpsimd.dma_start(out=msk_s, in_=msk_d[:, :, :])
        nc.gpsimd.dma_start(out=ones_s, in_=ones_d[:, :])

        # warm up the ACT exp table set before the real exps need it
        nc.vector.memset(warm[:, 0:4], 0.0)
        nc.scalar.activation(warm[:, 4:8], warm[:, 0:4], AF.Exp)

        # one shared PSUM pool, exactly 8 banks:
        #   s4a: [128,2,512] f32 = 2 banks x bufs 2   (heads 0-1 scores)
        #   s4b: [128,2,512] f32 = 2 banks x bufs 1   (heads 2-3 scores)
        #   y:   [128,512]  f32 = 1 bank              (PV accumulator)
        #   d:   [128,512]  f32 = 1 bank              (denominator accumulator)
        # proj tiles borrow tag s4a; out-proj tiles borrow tags y/d.
        ps = stack.enter_context(tc.tile_pool(name="ps", bufs=1, space="PSUM"))
        es = stack.enter_context(tc.tile_pool(name="es", bufs=1))
        rr = stack.enter_context(tc.tile_pool(name="rr", bufs=2))
        ot = stack.enter_context(tc.tile_pool(name="ot", bufs=3))

        def emit_proj_qk(n):
            n0, nn = Q_TILES[n]
            for m in range(2):
                qp = ps.tile([128, 2, 512], f32, name="qp", tag="s4a", bufs=2)
                for kk in range(2):
                    nc.tensor.matmul(
                        out=qp[:, 0, 0:nn],
                        lhsT=wqt[kk][:, m * 128 : (m + 1) * 128],
                        rhs=xt[kk][:, n0 : n0 + nn],
                        start=(kk == 0),
                        stop=(kk == 1),
                    )
                nc.vector.tensor_scalar_add(
                    out=qt[m][:, n0 : n0 + nn],
                    in0=qp[:, 0, 0:nn],
                    scalar1=bq_s[:, m : m + 1],
                )
                kp = ps.tile([128, 2, 512], f32, name="kp", tag="s4a", bufs=2)
                for kk in range(2):
                    nc.tensor.matmul(
                        out=kp[:, 0, 0:nn],
                        lhsT=wkt[kk][:, m * 128 : (m + 1) * 128],
                        rhs=xt[kk][:, n0 : n0 + nn],
                        start=(kk == 0),
                        stop=(kk == 1),
                    )
                nc.vector.tensor_scalar_add(
                    out=kt[m][:, n0 : n0 + nn],
                    in0=kp[:, 0, 0:nn],
                    scalar1=bk_s[:, m : m + 1],
                )

        def emit_proj_v(tts):
            for tt in tts:
                t0, tl = T_TILES[tt]
                vp = ps.tile([128, 2, 512], f32, name="vp", tag="s4a", bufs=2)
                for kk in range(2):
                    nc.tensor.matmul(
                        out=vp[0:tl, 0, 0:C],
                        lhsT=xt[kk][:, t0 : t0 + tl],
                        rhs=wvt[kk][:, :],
                        start=(kk == 0),
                        stop=(kk == 1),
                    )
                nc.vector.tensor_tensor(
                    out=vnat[0:tl, tt * C : (tt + 1) * C],
                    in0=vp[0:tl, 0, 0:C],
                    in1=bv_s[0:tl, :],
                    op=ALU.add,
                )

        def emit_attn(qi, g):
            q0, qn = Q_TILES[qi]
            y_ps = ps.tile([128, 512], f32, name="y_ps", tag="y", bufs=1)
            d_ps = ps.tile([128, 512], f32, name="d_ps", tag="d", bufs=1)
            # zero data; accumulating matmuls use start=False so stale
            # has_written bits can't drop contributions either way.
            nc.vector.memset(y_ps[:, 0:qn], 0.0)
            nc.vector.memset(d_ps[:, 0:qn], 0.0)

            js = [j for j, (k0, kn) in enumerate(K_TILES) if k0 <= q0 + qn - 1]
            jlast = js[-1]

            def emit_S(j):
                k0, kn = K_TILES[j]
                r = max(0, k0 - q0)
                s4a = ps.tile([128, 2, 512], f32, name="s4a", tag="s4a", bufs=2)
                s4b = ps.tile([128, 2, 512], f32, name="s4b", tag="s4b", bufs=1)
                for hh in range(4):
                    dst = s4a if hh < 2 else s4b
                    nc.tensor.matmul(
                        out=dst[0:kn, hh % 2, r:qn],
                        lhsT=kt[g][32 * hh : 32 * (hh + 1), k0 : k0 + kn],
                        rhs=qt[g][32 * hh : 32 * (hh + 1), q0 + r : q0 + qn],
                        start=True,
                        stop=True,
                        tile_position=(32 * hh, 0),
                    )
                return s4a, s4b

            cur = emit_S(js[0])
            for idx, j in enumerate(js):
                k0, kn = K_TILES[j]
                r = max(0, k0 - q0)
                nxt = emit_S(js[idx + 1]) if idx + 1 < len(js) else None
                s4a, s4b = cur
                esl_a = es.tile([128, 2, 512], f32, name="esl_a", tag="esl_a", bufs=3)
                esl_b = es.tile([128, 2, 512], f32, name="esl_b", tag="esl_b", bufs=3)
                nc.scalar.activation(
                    out=esl_a[0:kn, :, r:qn], in_=s4a[0:kn, :, r:qn],
                    func=AF.Exp, scale=SCALE,
                )
                nc.scalar.activation(
                    out=esl_b[0:kn, :, r:qn], in_=s4b[0:kn, :, r:qn],
                    func=AF.Exp, scale=SCALE,
                )
                if k0 >= q0:  # diagonal-crossing block: 0/1 mask strip
                    w = min(kn, qn - r)
                    for esl in (esl_a, esl_b):
                        nc.vector.tensor_tensor(
                            out=esl[0:kn, :, r : r + w],
                            in0=esl[0:kn, :, r : r + w],
                            in1=msk_s[0:kn, :, 0:w],
                            op=ALU.mult,
                        )
                for hh in range(4):
                    esl = esl_a if hh < 2 else esl_b
                    rhs = esl[0:kn, hh % 2, r:qn]
                    nc.tensor.matmul(
                        out=y_ps[32 * hh : 32 * (hh + 1), r:qn],
                        lhsT=vnat[0:kn, j * C + g * 128 + 32 * hh : j * C + g * 128 + 32 * (hh + 1)],
                        rhs=rhs,
                        start=False,
                        stop=(j == jlast),
                        tile_position=(0, 32 * hh),
                        skip_group_check=True,
                    )
                    nc.tensor.matmul(
                        out=d_ps[32 * hh : 32 * (hh + 1), r:qn],
                        lhsT=ones_s[0:kn, :],
                        rhs=rhs,
                        start=False,
                        stop=(j == jlast),
                        tile_position=(0, 32 * hh),
                        skip_group_check=True,
                    )
                cur = nxt
            rt = rr.tile([128, 512], f32, name="rt", tag="rt")
            nc.vector.reciprocal_approx_fast(out=rt[:, 0:qn], in_=d_ps[:, 0:qn])
            nc.vector.tensor_tensor(
                out=yt[g][:, q0 : q0 + qn],
                in0=y_ps[:, 0:qn],
                in1=rt[:, 0:qn],
                op=ALU.mult,
            )

        # interleave projections with attention so proj matmuls backfill
        # PE idle slots while ACT is busy with exp
        emit_proj_qk(0)
        emit_proj_v(range(0, 4))
        emit_attn(0, 0)
        emit_attn(0, 1)
        emit_proj_qk(1)
        emit_proj_v(range(4, 8))
        emit_attn(1, 0)
        emit_attn(1, 1)
        emit_proj_qk(2)
        emit_proj_v(range(8, 12))
        emit_attn(2, 0)
        emit_attn(2, 1)

        # ---------------- output projection tail ----------------
        for i, (t0, tl) in enumerate(T_TILES):
            ops = ps.tile(
                [128, 512], f32, name="ops", tag=("y" if i % 2 == 0 else "d"), bufs=1
            )
            nc.tensor.matmul(
                out=ops[0:tl, 0:C],
                lhsT=yt[0][:, t0 : t0 + tl],
                rhs=wpt[0][:, :],
                start=True,
                stop=False,
            )
            nc.tensor.matmul(
                out=ops[0:tl, 0:C],
                lhsT=yt[1][:, t0 : t0 + tl],
                rhs=wpt[1][:, :],
                start=False,
                stop=True,
            )
            ost = ot.tile([128, C], f32, name="ost", tag="ost")
            nc.vector.tensor_tensor(
                out=ost[0:tl, :],
                in0=ops[0:tl, 0:C],
                in1=bp_s[0:tl, :],
                op=ALU.add,
            )
            nc.sync.dma_start(out=out_d[t0 : t0 + tl, :], in_=ost[0:tl, :])

    nc.compile()
    return nc


def _get_nc():
    if "nc" not in _CACHE:
        _CACHE["nc"] = _build()
    return _CACHE["nc"]


def _make_in_maps(inputs):
    f = np.float32
    x = np.asarray(inputs["x"], f)
    Wq = np.asarray(inputs["Wq"], f)
    Wk = np.asarray(inputs["Wk"], f)
    Wv = np.asarray(inputs["Wv"], f)
    Wp = np.asarray(inputs["Wp"], f)
    bq = np.asarray(inputs["bq"], f)
    bk = np.asarray(inputs["bk"], f)
    bv = np.asarray(inputs["bv"], f)
    bp = np.asarray(inputs["bp"], f)

    tri = np.triu(np.ones((128, 128), f))  # keep where k-row <= q-col
    common = {
        "wqt": np.ascontiguousarray(Wq.T),
        "wkt": np.ascontiguousarray(Wk.T),
        "wvt": np.ascontiguousarray(Wv.T),
        "wpt": np.ascontiguousarray(Wp.T),
        "bq2": np.ascontiguousarray(bq.reshape(2, 128).T),
        "bk2": np.ascontiguousarray(bk.reshape(2, 128).T),
        "bvrep": np.ascontiguousarray(np.tile(bv, (128, 1))),
        "bprep": np.ascontiguousarray(np.tile(bp, (128, 1))),
        "bmask2": np.ascontiguousarray(np.tile(tri[:, None, :], (1, 2, 1))),
        "ones32": np.ones((128, 32), f),
    }
    return [
        {**common, "xt": np.ascontiguousarray(x[b].T)} for b in range(N_CORES)
    ]


def run(inputs, trace=False):
    from concourse.bass_utils import run_bass_kernel_spmd

    nc = _get_nc()
    in_maps = _make_in_maps(inputs)
    res = run_bass_kernel_spmd(nc, in_maps, list(range(N_CORES)), trace=trace)
    out = np.stack([res.results[i]["out"] for i in range(N_CORES)], axis=0)
    return out.astype(np.float32), res


def kernel(**inputs) -> np.ndarray:
    out, _ = run(inputs, trace=False)
    return out
